# revision 44
# baseline (speedup 1.0000x reference)
"""Trainium2 Bass kernel for nn_PairedKidneyBackbone (GAT message passing).

Strategy: cores 0-3 -> graph 0, cores 4-7 -> graph 1; each core owns a
quarter (2500) of its graph's dst nodes.  Per GAT layer:
  A. h = x@W from x^T (channels-on-partitions) with fused score columns.
     haug = [h | s_src | pad] bf16 rows written to HBM (L0: [x0 | s_src]).
  B. dma_gather of haug rows by dst-sorted src index; second gather of fp32
     s_dst rows; ex = exp(leakyrelu(s_src+s_dst)); per 128-edge chunk a
     "staircase" lhsT[p,m] = ex[p]*(dstloc[p]==m) built by one DVE op, then
     PE matmuls accumulate psum[128 nodes, C] and a denominator column ->
     softmax aggregation with no segment reductions.
  C. AllGather raw quarters within each graph's 4 cores (L0, L1 only).
  D. xbar-transpose readback to x^T, graph-LN (+folded conv bias) + ReLU via
     per-partition scale/bias on channels-on-partitions layout.
Final: residual + 2 FF layers + active mask per-quarter, host reassembles.
"""
import os
import sys

for p in ("/opt/trn_rl_repo", "/root/.axon_site", "/root/.axon_site/_ro/trn_rl_repo",
          "/root/.axon_site/_ro/pypackages"):
    if os.path.isdir(p) and p not in sys.path:
        sys.path.append(p)

import numpy as np
import ml_dtypes

B, N, H, H2 = 2, 10000, 256, 512
Q = N // 4              # 2500 dst nodes per core
NT = 20                 # quarter node tiles (last has 68 rows)
GT = 79                 # full-graph node tiles (padded to 10112 rows)
NPAD = GT * 128         # 10112
SC = 16                 # gather super-chunk size (chunks of 128 slots)
BF16 = ml_dtypes.bfloat16

_last_results = None    # test.py introspection
_last_nc = None
_last_in_maps = None
_last_nch = None
_last_consts = None


def _bf(x):
    return np.ascontiguousarray(np.asarray(x, np.float32).astype(BF16))


def _wrap16(idx):
    a = np.asarray(idx, np.int16)
    assert len(a) % 16 == 0
    w = a.reshape(-1, 16).T.copy()
    return np.tile(w, (8, 1))


def _plan_core(src_g, dst_g, q):
    """Edge plan for one core: dst in [q*Q,(q+1)*Q), dst-sorted, 128-padded
    per 128-node tile. Returns src_sorted, dst_sorted, dstloc, tile_nchunks."""
    lo, hi = q * Q, (q + 1) * Q
    sel = (dst_g >= lo) & (dst_g < hi)
    es, ed = src_g[sel], dst_g[sel]
    order = np.argsort(ed, kind="stable")
    es, ed = es[order], ed[order]
    bounds = np.searchsorted(ed, lo + 128 * np.arange(NT + 1))
    srcs, dstls, nch = [], [], []
    for t in range(NT):
        ts = es[bounds[t]:bounds[t + 1]]
        td = ed[bounds[t]:bounds[t + 1]] - (lo + t * 128)
        n = max(1, (len(ts) + 127) // 128)      # >=1 chunk per tile
        pad = n * 128 - len(ts)
        srcs.append(np.concatenate([ts, np.zeros(pad, np.int64)]))
        dstls.append(np.concatenate([td, np.full(pad, 999, np.int64)]))
        nch.append(n)
    return srcs, dstls, nch


def _build_plans(src, dst):
    """Per-core plans with chunk counts unified across the 8 cores."""
    percore = []
    for g in range(B):
        m = (dst >= g * N) & (dst < (g + 1) * N)
        src_g, dst_g = src[m] - g * N, dst[m] - g * N
        assert (src_g >= 0).all() and (src_g < N).all(), "cross-graph edge"
        for q in range(4):
            percore.append(_plan_core(src_g, dst_g, q))
    nch = [max(percore[c][2][t] for c in range(8)) for t in range(NT)]
    plans = []
    for c in range(8):
        srcs, dstls = [], []
        for t in range(NT):
            pad = (nch[t] - percore[c][2][t]) * 128
            srcs.append(np.concatenate(
                [percore[c][0][t], np.zeros(pad, np.int64)]))
            dstls.append(np.concatenate(
                [percore[c][1][t], np.full(pad, 999, np.int64)]))
        src_sorted = np.concatenate(srcs)
        dstloc = np.concatenate(dstls)
        tile_base = np.repeat(np.arange(NT) * 128, np.array(nch) * 128)
        dst_sorted = np.where(dstloc == 999, 0,
                              dstloc + tile_base + (c % 4) * Q)
        plans.append(dict(src_sorted=src_sorted, dst_sorted=dst_sorted,
                          dstloc=dstloc))
    return plans, nch


def _build_program(nch, caugs, consts, single_core=False, level=99, no_sd=False, no_den=False):
    import concourse.bacc as bacc
    import concourse.mybir as mybir
    import concourse.tile as tile
    from concourse import masks
    from concourse.library_config import mlp

    FP32, BF = mybir.dt.float32, mybir.dt.bfloat16
    I16, I32 = mybir.dt.int16, mybir.dt.int32
    AL, AF = mybir.AluOpType, mybir.ActivationFunctionType
    NCHT = sum(nch)
    SLOTS = NCHT * 128
    CAUG0, CAUG1, CAUG2 = caugs

    nc = bacc.Bacc("TRN2", target_bir_lowering=False, debug=False,
                   enable_asserts=True, num_devices=1 if single_core else 8)

    def din(name, shape, dt):
        return nc.dram_tensor(name, shape, dt, kind="ExternalInput")

    d_nfT = din("nfT", [8, NPAD], BF)          # [nf^T ; ones], zero-padded
    d_nfqT = din("nfqT", [8, NT * 128], BF)    # quarter slice of nfT
    d_emb = din("embrhs", [8, 258], BF)        # [[Wemb|Wemb@wsd0];[bias row]]
    d_embq = din("embqrow", [8, 256], BF)      # Wemb with bemb+g2_b ones-row
    d_W1 = din("W1", [512, 512], BF)
    d_W2 = din("W2", [512, 256], BF)
    d_wsd1 = din("wsd1", [512, 2], BF)
    d_wsd2 = din("wsd2", [512, 2], BF)
    d_g0W = din("g0W", [256, 512], BF)         # L0 post-aggregation matmul
    d_ln = din("ln", [128, 4 * 6], FP32)       # lnw0|lnb0|gb0|lnw1|lnb1|gb1
    d_ffW = din("ffW", [512, 256], BF)         # ff1_W ; ff2_W stacked
    d_ffrow = din("ffrow", [1, 512], BF)       # ff1_b | ff2_b
    d_isrc = din("isrc", [128, SLOTS // 16], I16)
    d_idst = din("idst", [128, SLOTS // 16], I16)
    d_dstloc = din("dstloc", [128, NCHT], FP32)
    d_act = din("actq", [128, NT], FP32)
    d_out = nc.dram_tensor("out", [NT * 128, 256], FP32, kind="ExternalOutput")

    with tile.TileContext(nc) as tc:
        with (
            tc.tile_pool(name="const", bufs=1) as cp,
            tc.tile_pool(name="wp", bufs=1) as wp,
            tc.tile_pool(name="xt", bufs=1) as xtp,
            tc.tile_pool(name="gath", bufs=2) as gp,
            tc.tile_pool(name="sdg", bufs=2) as sdp,
            tc.tile_pool(name="wk", bufs=2) as wk,
            tc.tile_pool(name="st", bufs=4) as stp,
            tc.tile_pool(name="oq", bufs=1) as oqp,
            tc.tile_pool(name="dram", bufs=1, space="DRAM") as dram,
            tc.tile_pool(name="ps_big", bufs=2, space="PSUM") as psb,
            tc.tile_pool(name="ps_sm", bufs=2, space="PSUM") as pss,
            tc.tile_pool(name="ps_ag", bufs=2, space="PSUM") as psa,
            tc.tile_pool(name="ps_t", bufs=2, space="PSUM") as pst,
        ):
            nc.gpsimd.load_library(mlp)
            # ---- constants ----
            iota_i = cp.tile([128, 128], I32)
            nc.gpsimd.iota(iota_i[:], pattern=[[1, 128]], base=0,
                           channel_multiplier=0)
            iota_f = cp.tile([128, 128], FP32)
            nc.vector.tensor_copy(iota_f[:], iota_i[:])
            ident = cp.tile([128, 128], BF)
            masks.make_identity(nc, ident[:])
            e0 = cp.tile([128, 128], BF)
            nc.vector.memset(e0[:], 0.0)
            nc.vector.memset(e0[:1, :], 1.0)
            ones_col = cp.tile([128, 1], BF)
            nc.vector.memset(ones_col[:], 1.0)
            ones_colf = cp.tile([128, 1], FP32)
            nc.vector.memset(ones_colf[:], 1.0)
            ones_row = cp.tile([1, 128], FP32)
            nc.vector.memset(ones_row[:], 1.0)
            isrc = cp.tile([128, SLOTS // 16], I16)
            idst = cp.tile([128, SLOTS // 16], I16)
            dstloc = cp.tile([128, NCHT], FP32)
            actq = cp.tile([128, NT], FP32)
            lnt = cp.tile([128, 24], FP32)
            nc.sync.dma_start(isrc[:], d_isrc[:, :])
            nc.sync.dma_start(idst[:], d_idst[:, :])
            nc.sync.dma_start(dstloc[:], d_dstloc[:, :])
            nc.sync.dma_start(actq[:], d_act[:, :])
            nc.sync.dma_start(lnt[:], d_ln[:, :])
            ffrow = cp.tile([128, 512], BF)
            nc.vector.memset(ffrow[:], 0.0)
            nc.sync.dma_start(ffrow[:1, :], d_ffrow[:, :])
            g0W = cp.tile([128, 2, 512], BF)
            nc.sync.dma_start(g0W[:], d_g0W.rearrange("(a p) c -> p a c", p=128))
            ffW = cp.tile([128, 4, 256], BF)
            nc.sync.dma_start(ffW[:], d_ffW.rearrange("(a p) c -> p a c", p=128))

            # internal DRAM
            haug0 = dram.tile([NPAD, CAUG0], BF)
            haug1 = dram.tile([NPAD, CAUG1], BF)
            haug2 = dram.tile([NPAD, CAUG2], BF)
            sdarr = dram.tile([NPAD, 64], FP32)
            coll_in = [dram.tile([Q, 512], BF, name=f"cin{i}", tag=f"cin{i}")
                       for i in range(2)]
            coll_out = [dram.tile([N, 512], BF, name=f"cout{i}", tag=f"cout{i}")
                        for i in range(2)]

            xT = xtp.tile([128, 4, NPAD], BF)   # x^T planes, reused per layer
            x0q = oqp.tile([128, NT, 256], BF)  # residual base quarter
            out_q = oqp.tile([128, NT, 256], BF)

            # ---------- embedding phase ----------
            # full graph: haug0 rows = [x0 | s_src0 | pad], sdarr col0 = s_dst0
            if True:
                nfT = gp.tile([8, NPAD], BF, tag="G")
                nc.sync.dma_start(nfT[:], d_nfT[:, :])
                nfqT = gp.tile([8, NT * 128], BF, tag="G")
                nc.sync.dma_start(nfqT[:], d_nfqT[:, :])
                embrhs = wk.tile([8, 258], BF, tag="embrhs")
                nc.sync.dma_start(embrhs[:], d_emb[:, :])
                embqrhs = wk.tile([8, 256], BF, tag="embqrhs")
                nc.sync.dma_start(embqrhs[:], d_embq[:, :])
                for g4 in range(0, GT, 4):
                    nt4 = min(4, GT - g4)
                    hsb = wk.tile([128, 4, CAUG0], BF, tag="hsb")
                    sdw = wk.tile([128, 4, 64], FP32, tag="sdw")
                    for i in range(nt4):
                        gt = g4 + i
                        ps = psb.tile([128, 512], FP32, tag="psA")
                        nc.tensor.matmul(ps[:, :258],
                                         nfT[:, gt * 128:(gt + 1) * 128],
                                         embrhs[:], start=True, stop=True)
                        if i % 2 == 0:
                            nc.scalar.activation(hsb[:, i, :257], ps[:, :257],
                                                 AF.Copy)
                        else:
                            nc.vector.tensor_copy(hsb[:, i, :257], ps[:, :257])
                        nc.vector.tensor_copy(sdw[:, i, 0:1], ps[:, 257:258])
                    nc.sync.dma_start(
                        haug0[g4 * 128:(g4 + nt4) * 128, :].rearrange(
                            "(a p) c -> p a c", p=128), hsb[:, :nt4, :])
                    nc.sync.dma_start(
                        sdarr[g4 * 128:(g4 + nt4) * 128, :].rearrange(
                            "(a p) c -> p a c", p=128), sdw[:, :nt4, :])
                # quarter residual base: x0q = nfq@Wemb + (bemb + g2_b)
                for t in range(NT):
                    ps = psb.tile([128, 512], FP32, tag="psA")
                    nc.tensor.matmul(ps[:, :256],
                                     nfqT[:, t * 128:(t + 1) * 128],
                                     embqrhs[:], start=True, stop=True)
                    nc.scalar.activation(x0q[:, t, :], ps[:, :256], AF.Copy)

            # ---------- helper: aggregation phase ----------
            def agg_phase(haug, caug, cval, post_tile):
                """Gather+staircase aggregation over the quarter.
                post_tile(t, ps_agg, rec) emits out_q[:, t, :]."""
                tile_of = np.repeat(np.arange(NT), nch)
                first = np.concatenate([[True], np.diff(tile_of) != 0])
                last = np.append(first[1:], True)
                c = 0
                while c < NCHT:
                    w = min(SC, NCHT - c)
                    nidx = w * 128
                    G = gp.tile([128, SC, caug], BF, tag="G")
                    nc.gpsimd.dma_gather(
                        G[:, :w, :], haug[:, :], isrc[:, c * 8:(c + w) * 8],
                        nidx, nidx, caug, single_packet=False)
                    z = wk.tile([128, SC], FP32, tag="z", bufs=3)
                    if not no_sd:
                        SD = sdp.tile([128, SC, 64], FP32, tag="SD")
                        nc.gpsimd.dma_gather(
                            SD[:, :w, :], sdarr[:, :],
                            idst[:, c * 8:(c + w) * 8],
                            nidx, nidx, 64, single_packet=False)
                        nc.vector.tensor_tensor(
                            out=z[:, :w], in0=G[:, :w, cval],
                            in1=SD[:, :w, 0], op=AL.add)
                    else:
                        nc.vector.tensor_copy(z[:, :w], G[:, :w, cval])
                    nc.vector.scalar_tensor_tensor(
                        out=z[:, :w], in0=z[:, :w], scalar=0.2, in1=z[:, :w],
                        op0=AL.mult, op1=AL.max)
                    ex = wk.tile([128, SC], FP32, tag="ex", bufs=3)
                    nc.scalar.activation(ex[:, :w], z[:, :w], AF.Exp)
                    for j in range(w):
                        cc = c + j
                        if first[cc]:
                            ps_agg = psa.tile([128, 512], FP32, tag="psAG")
                            psd = pss.tile([128, 2], FP32, tag="psD")
                        stair = stp.tile([128, 128], BF, tag="stair")
                        nc.vector.tensor_scalar(
                            out=stair[:], in0=iota_f[:],
                            scalar1=dstloc[:, cc:cc + 1], scalar2=ex[:, j:j + 1],
                            op0=AL.is_equal, op1=AL.mult)
                        nc.tensor.matmul(ps_agg[:, :cval], stair[:],
                                         G[:, j, :cval], start=first[cc],
                                         stop=last[cc])
                        if not no_den:
                            nc.tensor.matmul(psd[:, :1], stair[:],
                                             ones_col[:], start=first[cc],
                                             stop=last[cc])
                        if last[cc]:
                            t = tile_of[cc]
                            den = wk.tile([128, 1], FP32, tag="den", bufs=4)
                            nc.vector.tensor_scalar(
                                out=den[:], in0=ps_agg[:, :1] if no_den
                                else psd[:, :1], scalar1=1e-30,
                                scalar2=None, op0=AL.add)
                            rec = wk.tile([128, 1], FP32, tag="rec", bufs=4)
                            nc.vector.reciprocal(rec[:], den[:])
                            post_tile(t, ps_agg, rec)
                    c += w

            # ---------- helper: exchange + LN readback ----------
            def exchange(li, cout):
                ci, co = coll_in[li], coll_out[li]
                if single_core:
                    # timing stand-in for the AllGather: 4 local HBM copies
                    for qq in range(4):
                        nc.sync.dma_start(co[qq * Q:(qq + 1) * Q, :cout],
                                          ci[:, :cout])
                else:
                    nc.gpsimd.collective_compute(
                        "AllGather", mybir.AluOpType.bypass,
                        replica_groups=[[0, 1, 2, 3], [4, 5, 6, 7]],
                        ins=[ci.opt()], outs=[co.opt()])
                return co

            def readback_ln(li, co, cout, lni):
                """Transpose-read coll_out into xT planes; graph-LN (+conv
                bias fold) + relu in place."""
                planes = cout // 128
                for p in range(planes):
                    nc.sync.dma_start(xT[:, p, :N],
                                      co[:, p * 128:(p + 1) * 128],
                                      transpose=True)
                    nc.vector.memset(xT[:, p, N:], 0.0)
                lnw = lnt[:, 12 * lni + 0:12 * lni + 4]
                lnb = lnt[:, 12 * lni + 4:12 * lni + 8]
                gb = lnt[:, 12 * lni + 8:12 * lni + 12]
                s1 = wk.tile([128, 4], FP32, tag="s1")
                s2 = wk.tile([128, 4], FP32, tag="s2")
                dummy = gp.tile([128, N], BF, tag="G")  # reuse a G slot
                for p in range(planes):
                    nc.scalar.activation(dummy[:], xT[:, p, :N], AF.Copy,
                                         accum_out=s1[:, p:p + 1])
                    nc.scalar.activation(dummy[:], xT[:, p, :N], AF.Square,
                                         accum_out=s2[:, p:p + 1])
                # per-partition partials: s1r, s2r, gb.s1
                sc3 = wk.tile([128, 3], FP32, tag="sc3")
                nc.vector.tensor_reduce(sc3[:, 0:1], s1[:, :planes],
                                        axis=mybir.AxisListType.X, op=AL.add)
                nc.vector.tensor_reduce(sc3[:, 1:2], s2[:, :planes],
                                        axis=mybir.AxisListType.X, op=AL.add)
                gbs1 = wk.tile([128, 4], FP32, tag="gbs1")
                nc.vector.tensor_tensor(out=gbs1[:, :planes], in0=gb[:, :planes],
                                        in1=s1[:, :planes], op=AL.mult)
                nc.vector.tensor_reduce(sc3[:, 2:3], gbs1[:, :planes],
                                        axis=mybir.AxisListType.X, op=AL.add)
                pr = pst.tile([1, 4], FP32, tag="psT")
                nc.tensor.matmul(pr[:, :3], ones_colf[:], sc3[:], start=True,
                                 stop=True)
                # scalar land  (consts baked per call)
                NG = consts[f"NG{lni}"]; NG2 = consts[f"NG2{lni}"]
                invNC = 1.0 / (N * cout)
                sc = wk.tile([1, 8], FP32, tag="scal")
                # mu = (S1 + NG) * invNC
                nc.vector.tensor_scalar(out=sc[:, 0:1], in0=pr[:, 0:1],
                                        scalar1=float(NG), scalar2=invNC,
                                        op0=AL.add, op1=AL.mult)
                # ms = S2 + NG2 + 2*gbs1
                nc.vector.tensor_scalar(out=sc[:, 1:2], in0=pr[:, 1:2],
                                        scalar1=float(NG2), scalar2=None,
                                        op0=AL.add)
                nc.vector.scalar_tensor_tensor(
                    out=sc[:, 1:2], in0=pr[:, 2:3], scalar=2.0, in1=sc[:, 1:2],
                    op0=AL.mult, op1=AL.add)
                # var = ms*invNC - mu^2 ; inv = 1/sqrt(var+eps)
                nc.vector.tensor_tensor(out=sc[:, 2:3], in0=sc[:, 0:1],
                                        in1=sc[:, 0:1], op=AL.mult)
                nc.vector.tensor_scalar(out=sc[:, 1:2], in0=sc[:, 1:2],
                                        scalar1=invNC, scalar2=None,
                                        op0=AL.mult)
                nc.vector.tensor_tensor(out=sc[:, 2:3], in0=sc[:, 1:2],
                                        in1=sc[:, 2:3], op=AL.subtract)
                nc.vector.tensor_scalar(out=sc[:, 2:3], in0=sc[:, 2:3],
                                        scalar1=1e-5, scalar2=None, op0=AL.add)
                nc.scalar.activation(sc[:, 3:4], sc[:, 2:3], AF.Sqrt)
                nc.vector.reciprocal(sc[:, 4:5], sc[:, 3:4])
                mi = wk.tile([1, 2], FP32, tag="mi")
                nc.vector.tensor_copy(mi[:, 0:1], sc[:, 0:1])
                nc.vector.tensor_copy(mi[:, 1:2], sc[:, 4:5])
                pb = pst.tile([128, 2], FP32, tag="psT")
                nc.tensor.matmul(pb[:], ones_row[:], mi[:], start=True,
                                 stop=True)
                mu_c = wk.tile([128, 2], FP32, tag="muc")
                nc.vector.tensor_copy(mu_c[:], pb[:])
                # per-plane scale/bias then relu pass
                for p in range(planes):
                    s_col = wk.tile([128, 1], FP32, tag="scol")
                    nc.vector.tensor_tensor(out=s_col[:], in0=mu_c[:, 1:2],
                                            in1=lnw[:, p:p + 1], op=AL.mult)
                    t_col = wk.tile([128, 1], FP32, tag="tcol")
                    nc.vector.tensor_tensor(out=t_col[:], in0=gb[:, p:p + 1],
                                            in1=mu_c[:, 0:1], op=AL.subtract)
                    nc.vector.tensor_tensor(out=t_col[:], in0=t_col[:],
                                            in1=s_col[:], op=AL.mult)
                    nc.vector.tensor_tensor(out=t_col[:], in0=t_col[:],
                                            in1=lnb[:, p:p + 1], op=AL.add)
                    nc.scalar.activation(xT[:, p, :], xT[:, p, :], AF.Relu,
                                         bias=t_col[:], scale=s_col[:])

            # ---------- L0 ----------
            def l0_post(t, ps_agg, rec):
                xbar = wk.tile([128, 256], BF, tag="xbar")
                nc.vector.tensor_scalar(out=xbar[:], in0=ps_agg[:, :256],
                                        scalar1=rec[:], scalar2=None,
                                        op0=AL.mult)
                psh = psb.tile([128, 512], FP32, tag="psA")
                for kc in range(2):
                    pt = pst.tile([128, 128], BF, tag="psT")
                    nc.tensor.transpose(pt[:], xbar[:, kc * 128:(kc + 1) * 128],
                                        ident[:])
                    xbT = wk.tile([128, 128], BF, tag="xbT")
                    nc.vector.tensor_copy(xbT[:], pt[:])
                    nc.tensor.matmul(psh[:], xbT[:], g0W[:, kc, :],
                                     start=(kc == 0), stop=(kc == 1))
                outt = wk.tile([128, 512], BF, tag="outt")
                nc.scalar.activation(outt[:], psh[:], AF.Copy)
                rows = 128 if t < NT - 1 else 68
                nc.sync.dma_start(
                    coll_in[0][t * 128:t * 128 + rows, :], outt[:rows, :])

            if level >= 1:
                agg_phase(haug0, CAUG0, 256, l0_post)
            if level >= 2:
                co = exchange(0, 512)
                readback_ln(0, co, 512, 0)

            # ---------- L1 / L2 phase A from xT ----------
            def phase_A(Wd, wsdd, cin, cout, haug, caug):
                Wsb = wp.tile([128, 4, 512], BF, tag="Wsb")
                nc.sync.dma_start(Wsb[:, :cin // 128, :cout],
                                  Wd.rearrange("(a p) c -> p a c", p=128))
                wsd = wp.tile([128, 4, 2], BF, tag="wsd")
                nc.sync.dma_start(wsd[:, :cin // 128, :],
                                  wsdd.rearrange("(a p) c -> p a c", p=128))
                kcs = cin // 128
                for g4 in range(0, GT, 4):
                    nt4 = min(4, GT - g4)
                    hsb = wk.tile([128, 4, 640], BF, tag="hsb")
                    sdw = wk.tile([128, 4, 64], FP32, tag="sdw")
                    for i in range(nt4):
                        gt = g4 + i
                        ps = psb.tile([128, 512], FP32, tag="psA")
                        ps2 = pss.tile([128, 2], FP32, tag="psD")
                        for kc in range(kcs):
                            lhsT = xT[:, kc, gt * 128:(gt + 1) * 128]
                            nc.tensor.matmul(ps[:, :cout], lhsT,
                                             Wsb[:, kc, :cout],
                                             start=(kc == 0),
                                             stop=(kc == kcs - 1))
                            nc.tensor.matmul(ps2[:], lhsT, wsd[:, kc, :],
                                             start=(kc == 0),
                                             stop=(kc == kcs - 1))
                        if i % 2 == 0:
                            nc.scalar.activation(hsb[:, i, :cout],
                                                 ps[:, :cout], AF.Copy)
                        else:
                            nc.vector.tensor_copy(hsb[:, i, :cout],
                                                  ps[:, :cout])
                        nc.vector.tensor_copy(hsb[:, i, cout:cout + 1],
                                              ps2[:, 0:1])
                        nc.vector.tensor_copy(sdw[:, i, 0:1], ps2[:, 1:2])
                    nc.sync.dma_start(
                        haug[g4 * 128:(g4 + nt4) * 128, :].rearrange(
                            "(a p) c -> p a c", p=128), hsb[:, :nt4, :caug])
                    nc.sync.dma_start(
                        sdarr[g4 * 128:(g4 + nt4) * 128, :].rearrange(
                            "(a p) c -> p a c", p=128), sdw[:, :nt4, :])

            def mk_post(cval, dest):
                def post(t, ps_agg, rec):
                    if dest is None:
                        nc.vector.tensor_scalar(
                            out=out_q[:, t, :cval], in0=ps_agg[:, :cval],
                            scalar1=rec[:], scalar2=None, op0=AL.mult)
                    else:
                        outt = wk.tile([128, 512], BF, tag="outt")
                        nc.vector.tensor_scalar(
                            out=outt[:, :cval], in0=ps_agg[:, :cval],
                            scalar1=rec[:], scalar2=None, op0=AL.mult)
                        rows = 128 if t < NT - 1 else 68
                        nc.sync.dma_start(
                            dest[t * 128:t * 128 + rows, :cval],
                            outt[:rows, :cval])
                return post

            if level >= 3:
                phase_A(d_W1, d_wsd1, 512, 512, haug1, CAUG1)
            if level >= 4:
                agg_phase(haug1, CAUG1, 512, mk_post(512, coll_in[1]))
            if level >= 5:
                co = exchange(1, 512)
                readback_ln(1, co, 512, 1)
            if level >= 6:
                phase_A(d_W2, d_wsd2, 512, 256, haug2, CAUG2)
            if level >= 7:
                agg_phase(haug2, CAUG2, 256, mk_post(256, None))
            # out_q[:, :, :256] now holds h2 quarter (no exchange)

            # ---------- final: residual + FF x2 + mask ----------
            if level < 8:
                # still produce the declared output so compile succeeds
                for t in range(NT):
                    outf = wk.tile([128, 256], FP32, tag="outf")
                    nc.vector.tensor_scalar(out=outf[:], in0=x0q[:, t, :],
                                            scalar1=actq[:, t:t + 1],
                                            scalar2=None, op0=AL.mult)
                    nc.sync.dma_start(d_out[t * 128:(t + 1) * 128, :], outf[:])
                nc.compile()
                return nc
            xf = oqp.tile([128, NT, 256], BF)
            for t in range(NT):
                nc.vector.tensor_tensor(out=xf[:, t, :], in0=x0q[:, t, :],
                                        in1=out_q[:, t, :256], op=AL.add)
            for ff in range(2):
                for t in range(NT):
                    psh = psb.tile([128, 512], FP32, tag="psA")
                    for kc in range(2):
                        pt = pst.tile([128, 128], BF, tag="psT")
                        nc.tensor.transpose(
                            pt[:], xf[:, t, kc * 128:(kc + 1) * 128], ident[:])
                        xbT = wk.tile([128, 128], BF, tag="xbT")
                        nc.vector.tensor_copy(xbT[:], pt[:])
                        nc.tensor.matmul(psh[:, :256], xbT[:],
                                         ffW[:, 2 * ff + kc, :],
                                         start=(kc == 0), stop=False)
                    nc.tensor.matmul(psh[:, :256], e0[:],
                                     ffrow[:, ff * 256:(ff + 1) * 256],
                                     start=False, stop=True)
                    relu = wk.tile([128, 256], BF, tag="relu")
                    nc.scalar.activation(relu[:], psh[:, :256], AF.Relu)
                    nc.vector.tensor_tensor(out=xf[:, t, :], in0=xf[:, t, :],
                                            in1=relu[:], op=AL.add)
            for t in range(NT):
                outf = wk.tile([128, 256], FP32, tag="outf")
                nc.vector.tensor_scalar(out=outf[:], in0=xf[:, t, :],
                                        scalar1=actq[:, t:t + 1], scalar2=None,
                                        op0=AL.mult)
                nc.sync.dma_start(d_out[t * 128:(t + 1) * 128, :], outf[:])

    nc.compile()
    return nc


def kernel(**inputs):
    global _last_results, _last_nc, _last_in_maps
    from concourse.bass_utils import run_bass_kernel_spmd

    inp = {k: np.asarray(v) for k, v in inputs.items()}
    src = inp["src"].astype(np.int64)
    dst = inp["dst"].astype(np.int64)
    plans, nch = _build_plans(src, dst)
    caugs = (384, 640, 384)

    # host weight folds
    Wemb = inp["emb_W1"].astype(np.float64) @ inp["emb_W2"].astype(np.float64)
    bemb = (inp["emb_b1"].astype(np.float64) @ inp["emb_W2"].astype(np.float64)
            + inp["emb_b2"])
    ws0 = inp["g0_W"] @ inp["g0_as"]
    wd0 = inp["g0_W"] @ inp["g0_ad"]
    wnf = Wemb @ np.stack([ws0, wd0], 1).astype(np.float64)   # [7,2]
    bnf = bemb @ np.stack([ws0, wd0], 1).astype(np.float64)   # [2]
    wsd1 = np.stack([inp["g1_W"] @ inp["g1_as"], inp["g1_W"] @ inp["g1_ad"]], 1)
    wsd2 = np.stack([inp["g2_W"] @ inp["g2_as"], inp["g2_W"] @ inp["g2_ad"]], 1)

    consts = {}
    for li in (0, 1):
        gb = inp[f"g{li}_b"].astype(np.float64)
        consts[f"NG{li}"] = float(N * gb.sum())
        consts[f"NG2{li}"] = float(N * (gb * gb).sum())
    nc = _build_program(nch, caugs, consts)

    def pack_ln(li):
        cols = np.zeros((128, 12), np.float32)
        w = inp[f"ln{li}_w"]; b = inp[f"ln{li}_b"]; gb = inp[f"g{li}_b"]
        for p in range(4):
            cols[:, p] = w[p * 128:(p + 1) * 128]
            cols[:, 4 + p] = b[p * 128:(p + 1) * 128]
            cols[:, 8 + p] = gb[p * 128:(p + 1) * 128]
        return cols

    ln_all = np.concatenate([pack_ln(0), pack_ln(1)], 1)

    # per-core inputs
    nfg = [np.zeros((8, NPAD), np.float32) for _ in range(B)]
    for g in range(B):
        nfg[g][:7, :N] = inp["node_features"][g].T
        nfg[g][7, :N] = 1.0
    embrhs = np.concatenate(
        [np.concatenate([Wemb, wnf], 1),
         np.concatenate([bemb, bnf]).reshape(1, 258)], 0)      # [8,258]
    embq = np.concatenate(
        [Wemb, (bemb + inp["g2_b"].astype(np.float64)).reshape(1, 256)], 0)
    ffW = np.concatenate([inp["ff1_W"], inp["ff2_W"]], 0)      # [512,256]
    ffrow = np.concatenate([inp["ff1_b"], inp["ff2_b"]]).reshape(1, 512)

    in_maps = []
    for c in range(8):
        g, q = c // 4, c % 4
        pl = plans[c]
        actq = np.zeros((128, NT), np.float32)
        aq = inp["active_agents"][g][q * Q:(q + 1) * Q]
        for t in range(NT):
            seg = aq[t * 128:(t + 1) * 128]
            actq[:len(seg), t] = seg
        nfq = np.zeros((8, NT * 128), np.float32)
        nfq[:7, :Q] = inp["node_features"][g][q * Q:(q + 1) * Q].T
        nfq[7, :Q] = 1.0
        dl = pl["dstloc"].astype(np.float32).reshape(-1, 128).T.copy()
        in_maps.append({
            "nfT": _bf(nfg[g]), "nfqT": _bf(nfq),
            "embrhs": _bf(embrhs), "embqrow": _bf(embq),
            "W1": _bf(inp["g1_W"]), "W2": _bf(inp["g2_W"]),
            "wsd1": _bf(wsd1), "wsd2": _bf(wsd2), "g0W": _bf(inp["g0_W"]),
            "ln": np.ascontiguousarray(ln_all, np.float32).copy(),
            "ffW": _bf(ffW), "ffrow": _bf(ffrow),
            "isrc": _wrap16(pl["src_sorted"]), "idst": _wrap16(pl["dst_sorted"]),
            "dstloc": np.ascontiguousarray(dl),
            "actq": actq,
        })
    _last_nc, _last_in_maps = nc, in_maps
    globals()["_last_nch"], globals()["_last_consts"] = nch, consts
    res = run_bass_kernel_spmd(nc, in_maps, core_ids=list(range(8)))
    _last_results = res
    out = np.zeros((B, N, H), np.float32)
    for c in range(8):
        g, q = c // 4, c % 4
        out[g, q * Q:(q + 1) * Q] = res.results[c]["out"][:Q]
    return out, inp["active_agents"].astype(np.float32)


# revision 45
# speedup vs baseline: 1.0317x; 1.0317x over previous
"""Trainium2 Bass kernel for nn_PairedKidneyBackbone (GAT message passing).

Strategy: cores 0-3 -> graph 0, cores 4-7 -> graph 1; each core owns a
quarter (2500) of its graph's dst nodes.  Per GAT layer:
  A. h = x@W from x^T (channels-on-partitions) with fused score columns.
     haug = [h | s_src | pad] bf16 rows written to HBM (L0: [x0 | s_src]).
  B. dma_gather of haug rows by dst-sorted src index; second gather of fp32
     s_dst rows; ex = exp(leakyrelu(s_src+s_dst)); per 128-edge chunk a
     "staircase" lhsT[p,m] = ex[p]*(dstloc[p]==m) built by one DVE op, then
     PE matmuls accumulate psum[128 nodes, C] and a denominator column ->
     softmax aggregation with no segment reductions.
  C. AllGather raw quarters within each graph's 4 cores (L0, L1 only).
  D. xbar-transpose readback to x^T, graph-LN (+folded conv bias) + ReLU via
     per-partition scale/bias on channels-on-partitions layout.
Final: residual + 2 FF layers + active mask per-quarter, host reassembles.
"""
import os
import sys

for p in ("/opt/trn_rl_repo", "/root/.axon_site", "/root/.axon_site/_ro/trn_rl_repo",
          "/root/.axon_site/_ro/pypackages"):
    if os.path.isdir(p) and p not in sys.path:
        sys.path.append(p)

import numpy as np
import ml_dtypes

B, N, H, H2 = 2, 10000, 256, 512
Q = N // 4              # 2500 dst nodes per core
NT = 20                 # quarter node tiles (last has 68 rows)
GT = 79                 # full-graph node tiles (padded to 10112 rows)
NPAD = GT * 128         # 10112
SC = 16                 # gather super-chunk size (chunks of 128 slots)
BF16 = ml_dtypes.bfloat16

_last_results = None    # test.py introspection
_last_nc = None
_last_in_maps = None
_last_nch = None
_last_consts = None


def _bf(x):
    return np.ascontiguousarray(np.asarray(x, np.float32).astype(BF16))


def _wrap16(idx):
    a = np.asarray(idx, np.int16)
    assert len(a) % 16 == 0
    w = a.reshape(-1, 16).T.copy()
    return np.tile(w, (8, 1))


def _plan_core(src_g, dst_g, q):
    """Edge plan for one core: dst in [q*Q,(q+1)*Q), dst-sorted, 128-padded
    per 128-node tile. Returns src_sorted, dst_sorted, dstloc, tile_nchunks."""
    lo, hi = q * Q, (q + 1) * Q
    sel = (dst_g >= lo) & (dst_g < hi)
    es, ed = src_g[sel], dst_g[sel]
    order = np.argsort(ed, kind="stable")
    es, ed = es[order], ed[order]
    bounds = np.searchsorted(ed, lo + 128 * np.arange(NT + 1))
    srcs, dstls, nch = [], [], []
    for t in range(NT):
        ts = es[bounds[t]:bounds[t + 1]]
        td = ed[bounds[t]:bounds[t + 1]] - (lo + t * 128)
        n = max(1, (len(ts) + 127) // 128)      # >=1 chunk per tile
        pad = n * 128 - len(ts)
        srcs.append(np.concatenate([ts, np.zeros(pad, np.int64)]))
        dstls.append(np.concatenate([td, np.full(pad, 999, np.int64)]))
        nch.append(n)
    return srcs, dstls, nch


def _build_plans(src, dst):
    """Per-core plans with chunk counts unified across the 8 cores."""
    percore = []
    for g in range(B):
        m = (dst >= g * N) & (dst < (g + 1) * N)
        src_g, dst_g = src[m] - g * N, dst[m] - g * N
        assert (src_g >= 0).all() and (src_g < N).all(), "cross-graph edge"
        for q in range(4):
            percore.append(_plan_core(src_g, dst_g, q))
    nch = [max(percore[c][2][t] for c in range(8)) for t in range(NT)]
    plans = []
    for c in range(8):
        srcs, dstls = [], []
        for t in range(NT):
            pad = (nch[t] - percore[c][2][t]) * 128
            srcs.append(np.concatenate(
                [percore[c][0][t], np.zeros(pad, np.int64)]))
            dstls.append(np.concatenate(
                [percore[c][1][t], np.full(pad, 999, np.int64)]))
        src_sorted = np.concatenate(srcs)
        dstloc = np.concatenate(dstls)
        tile_base = np.repeat(np.arange(NT) * 128, np.array(nch) * 128)
        dst_sorted = np.where(dstloc == 999, 0,
                              dstloc + tile_base + (c % 4) * Q)
        plans.append(dict(src_sorted=src_sorted, dst_sorted=dst_sorted,
                          dstloc=dstloc))
    return plans, nch


def _build_program(nch, caugs, consts, single_core=False, level=99, no_sd=False, no_den=False):
    import concourse.bacc as bacc
    import concourse.mybir as mybir
    import concourse.tile as tile
    from concourse import masks
    from concourse.library_config import mlp

    FP32, BF = mybir.dt.float32, mybir.dt.bfloat16
    I16, I32 = mybir.dt.int16, mybir.dt.int32
    AL, AF = mybir.AluOpType, mybir.ActivationFunctionType
    NCHT = sum(nch)
    SLOTS = NCHT * 128
    CAUG0, CAUG1, CAUG2 = caugs

    nc = bacc.Bacc("TRN2", target_bir_lowering=False, debug=False,
                   enable_asserts=True, num_devices=1 if single_core else 8)

    def din(name, shape, dt):
        return nc.dram_tensor(name, shape, dt, kind="ExternalInput")

    d_nfT = din("nfT", [8, NPAD], BF)          # [nf^T ; ones], zero-padded
    d_nfqT = din("nfqT", [8, NT * 128], BF)    # quarter slice of nfT
    d_emb = din("embrhs", [8, 258], BF)        # [[Wemb|Wemb@wsd0];[bias row]]
    d_embq = din("embqrow", [8, 256], BF)      # Wemb with bemb+g2_b ones-row
    d_W1 = din("W1", [512, 512], BF)
    d_W2 = din("W2", [512, 256], BF)
    d_wsd1 = din("wsd1", [512, 2], BF)
    d_wsd2 = din("wsd2", [512, 2], BF)
    d_g0W = din("g0W", [256, 512], BF)         # L0 post-aggregation matmul
    d_ln = din("ln", [128, 4 * 6], FP32)       # lnw0|lnb0|gb0|lnw1|lnb1|gb1
    d_ffW = din("ffW", [512, 256], BF)         # ff1_W ; ff2_W stacked
    d_ffrow = din("ffrow", [1, 512], BF)       # ff1_b | ff2_b
    d_isrc = din("isrc", [128, SLOTS // 16], I16)
    d_idst = din("idst", [128, SLOTS // 16], I16)
    d_dstloc = din("dstloc", [128, NCHT], FP32)
    d_act = din("actq", [128, NT], FP32)
    d_out = nc.dram_tensor("out", [NT * 128, 256], FP32, kind="ExternalOutput")

    with tile.TileContext(nc) as tc:
        with (
            tc.tile_pool(name="const", bufs=1) as cp,
            tc.tile_pool(name="wp", bufs=1) as wp,
            tc.tile_pool(name="xt", bufs=1) as xtp,
            tc.tile_pool(name="gath", bufs=2) as gp,
            tc.tile_pool(name="sdg", bufs=2) as sdp,
            tc.tile_pool(name="wk", bufs=2) as wk,
            tc.tile_pool(name="st", bufs=4) as stp,
            tc.tile_pool(name="oq", bufs=1) as oqp,
            tc.tile_pool(name="dram", bufs=1, space="DRAM") as dram,
            tc.tile_pool(name="ps_big", bufs=2, space="PSUM") as psb,
            tc.tile_pool(name="ps_sm", bufs=2, space="PSUM") as pss,
            tc.tile_pool(name="ps_ag", bufs=2, space="PSUM") as psa,
            tc.tile_pool(name="ps_t", bufs=2, space="PSUM") as pst,
        ):
            nc.gpsimd.load_library(mlp)
            # ---- constants ----
            iota_i = cp.tile([128, 128], I32)
            nc.gpsimd.iota(iota_i[:], pattern=[[1, 128]], base=0,
                           channel_multiplier=0)
            iota_f = cp.tile([128, 128], FP32)
            nc.vector.tensor_copy(iota_f[:], iota_i[:])
            ident = cp.tile([128, 128], BF)
            masks.make_identity(nc, ident[:])
            e0 = cp.tile([128, 128], BF)
            nc.vector.memset(e0[:], 0.0)
            nc.vector.memset(e0[:1, :], 1.0)
            ones_col = cp.tile([128, 1], BF)
            nc.vector.memset(ones_col[:], 1.0)
            ones_colf = cp.tile([128, 1], FP32)
            nc.vector.memset(ones_colf[:], 1.0)
            ones_row = cp.tile([1, 128], FP32)
            nc.vector.memset(ones_row[:], 1.0)
            isrc = cp.tile([128, SLOTS // 16], I16)
            idst = cp.tile([128, SLOTS // 16], I16)
            dstloc = cp.tile([128, NCHT], FP32)
            actq = cp.tile([128, NT], FP32)
            lnt = cp.tile([128, 24], FP32)
            nc.sync.dma_start(isrc[:], d_isrc[:, :])
            nc.sync.dma_start(idst[:], d_idst[:, :])
            nc.sync.dma_start(dstloc[:], d_dstloc[:, :])
            nc.sync.dma_start(actq[:], d_act[:, :])
            nc.sync.dma_start(lnt[:], d_ln[:, :])
            ffrow = cp.tile([128, 512], BF)
            nc.vector.memset(ffrow[:], 0.0)
            nc.sync.dma_start(ffrow[:1, :], d_ffrow[:, :])
            g0W = cp.tile([128, 2, 512], BF)
            nc.sync.dma_start(g0W[:], d_g0W.rearrange("(a p) c -> p a c", p=128))
            ffW = cp.tile([128, 4, 256], BF)
            nc.sync.dma_start(ffW[:], d_ffW.rearrange("(a p) c -> p a c", p=128))

            # internal DRAM
            haug0 = dram.tile([NPAD, CAUG0], BF)
            haug1 = dram.tile([NPAD, CAUG1], BF)
            haug2 = dram.tile([NPAD, CAUG2], BF)
            sdarr = dram.tile([NPAD, 64], FP32)
            coll_in = [dram.tile([Q, 512], BF, name=f"cin{i}", tag=f"cin{i}")
                       for i in range(2)]
            coll_out = [dram.tile([N, 512], BF, name=f"cout{i}", tag=f"cout{i}")
                        for i in range(2)]

            xT = xtp.tile([128, 4, NPAD], BF)   # x^T planes, reused per layer
            x0q = oqp.tile([128, NT, 256], BF)  # residual base quarter
            out_q = oqp.tile([128, NT, 256], BF)

            # ---------- embedding phase ----------
            # full graph: haug0 rows = [x0 | s_src0 | pad], sdarr col0 = s_dst0
            if True:
                nfT = gp.tile([8, NPAD], BF, tag="G")
                nc.sync.dma_start(nfT[:], d_nfT[:, :])
                nfqT = gp.tile([8, NT * 128], BF, tag="G")
                nc.sync.dma_start(nfqT[:], d_nfqT[:, :])
                embrhs = wk.tile([8, 258], BF, tag="embrhs")
                nc.sync.dma_start(embrhs[:], d_emb[:, :])
                embqrhs = wk.tile([8, 256], BF, tag="embqrhs")
                nc.sync.dma_start(embqrhs[:], d_embq[:, :])
                for g4 in range(0, GT, 4):
                    nt4 = min(4, GT - g4)
                    hsb = wk.tile([128, 4, CAUG0], BF, tag="hsb")
                    sdw = wk.tile([128, 4, 64], FP32, tag="sdw")
                    for i in range(nt4):
                        gt = g4 + i
                        ps = psb.tile([128, 512], FP32, tag="psA")
                        nc.tensor.matmul(ps[:, :258],
                                         nfT[:, gt * 128:(gt + 1) * 128],
                                         embrhs[:], start=True, stop=True)
                        if i % 2 == 0:
                            nc.scalar.activation(hsb[:, i, :257], ps[:, :257],
                                                 AF.Copy)
                        else:
                            nc.vector.tensor_copy(hsb[:, i, :257], ps[:, :257])
                        nc.vector.tensor_copy(sdw[:, i, 0:1], ps[:, 257:258])
                    nc.sync.dma_start(
                        haug0[g4 * 128:(g4 + nt4) * 128, :].rearrange(
                            "(a p) c -> p a c", p=128), hsb[:, :nt4, :])
                    nc.sync.dma_start(
                        sdarr[g4 * 128:(g4 + nt4) * 128, :].rearrange(
                            "(a p) c -> p a c", p=128), sdw[:, :nt4, :])
                # quarter residual base: x0q = nfq@Wemb + (bemb + g2_b)
                for t in range(NT):
                    ps = psb.tile([128, 512], FP32, tag="psA")
                    nc.tensor.matmul(ps[:, :256],
                                     nfqT[:, t * 128:(t + 1) * 128],
                                     embqrhs[:], start=True, stop=True)
                    nc.scalar.activation(x0q[:, t, :], ps[:, :256], AF.Copy)

            # ---------- helper: aggregation phase ----------
            def agg_phase(haug, caug, cval, post_tile):
                """Gather+staircase aggregation over the quarter.
                post_tile(t, ps_agg, rec) emits out_q[:, t, :]."""
                tile_of = np.repeat(np.arange(NT), nch)
                first = np.concatenate([[True], np.diff(tile_of) != 0])
                last = np.append(first[1:], True)
                c = 0
                while c < NCHT:
                    w = min(SC, NCHT - c)
                    nidx = w * 128
                    G = gp.tile([128, SC, caug], BF, tag="G")
                    nc.gpsimd.dma_gather(
                        G[:, :w, :], haug[:, :], isrc[:, c * 8:(c + w) * 8],
                        nidx, nidx, caug, single_packet=False)
                    z = wk.tile([128, SC], FP32, tag="z", bufs=3)
                    if not no_sd:
                        SD = sdp.tile([128, SC, 64], FP32, tag="SD")
                        nc.gpsimd.dma_gather(
                            SD[:, :w, :], sdarr[:, :],
                            idst[:, c * 8:(c + w) * 8],
                            nidx, nidx, 64, single_packet=False)
                        nc.vector.tensor_tensor(
                            out=z[:, :w], in0=G[:, :w, cval],
                            in1=SD[:, :w, 0], op=AL.add)
                    else:
                        nc.vector.tensor_copy(z[:, :w], G[:, :w, cval])
                    nc.vector.scalar_tensor_tensor(
                        out=z[:, :w], in0=z[:, :w], scalar=0.2, in1=z[:, :w],
                        op0=AL.mult, op1=AL.max)
                    ex = wk.tile([128, SC], FP32, tag="ex", bufs=3)
                    nc.scalar.activation(ex[:, :w], z[:, :w], AF.Exp)
                    for j in range(w):
                        cc = c + j
                        if first[cc]:
                            ps_agg = psa.tile([128, 512], FP32, tag="psAG")
                            psd = pss.tile([128, 2], FP32, tag="psD")
                        stair = stp.tile([128, 128], BF, tag="stair")
                        nc.vector.tensor_scalar(
                            out=stair[:], in0=iota_f[:],
                            scalar1=dstloc[:, cc:cc + 1], scalar2=ex[:, j:j + 1],
                            op0=AL.is_equal, op1=AL.mult)
                        nc.tensor.matmul(ps_agg[:, :cval], stair[:],
                                         G[:, j, :cval], start=first[cc],
                                         stop=last[cc])
                        if not no_den:
                            nc.tensor.matmul(psd[:, :1], stair[:],
                                             ones_col[:], start=first[cc],
                                             stop=last[cc])
                        if last[cc]:
                            t = tile_of[cc]
                            den = wk.tile([128, 1], FP32, tag="den", bufs=4)
                            nc.vector.tensor_scalar(
                                out=den[:], in0=ps_agg[:, :1] if no_den
                                else psd[:, :1], scalar1=1e-30,
                                scalar2=None, op0=AL.add)
                            rec = wk.tile([128, 1], FP32, tag="rec", bufs=4)
                            nc.vector.reciprocal(rec[:], den[:])
                            post_tile(t, ps_agg, rec)
                    c += w

            # ---------- helper: exchange + LN readback ----------
            def exchange(li, cout):
                ci, co = coll_in[li], coll_out[li]
                if single_core:
                    # timing stand-in for the AllGather: 4 local HBM copies
                    for qq in range(4):
                        nc.sync.dma_start(co[qq * Q:(qq + 1) * Q, :cout],
                                          ci[:, :cout])
                else:
                    nc.gpsimd.collective_compute(
                        "AllGather", mybir.AluOpType.bypass,
                        replica_groups=[[0, 1, 2, 3], [4, 5, 6, 7]],
                        ins=[ci.opt()], outs=[co.opt()])
                return co

            def readback_ln(li, co, cout, lni):
                """Transpose-read coll_out into xT planes; graph-LN (+conv
                bias fold) + relu in place."""
                planes = cout // 128
                for p in range(planes):
                    nc.sync.dma_start(xT[:, p, :N],
                                      co[:, p * 128:(p + 1) * 128],
                                      transpose=True)
                    nc.vector.memset(xT[:, p, N:], 0.0)
                lnw = lnt[:, 12 * lni + 0:12 * lni + 4]
                lnb = lnt[:, 12 * lni + 4:12 * lni + 8]
                gb = lnt[:, 12 * lni + 8:12 * lni + 12]
                s1 = wk.tile([128, 4], FP32, tag="s1")
                s2 = wk.tile([128, 4], FP32, tag="s2")
                dummy = gp.tile([128, N], BF, tag="G")  # reuse a G slot
                for p in range(planes):
                    if p < planes // 2:
                        nc.scalar.activation(dummy[:], xT[:, p, :N], AF.Copy,
                                             accum_out=s1[:, p:p + 1])
                    else:
                        # native DVE reduce, parallel with ACT's passes
                        nc.vector.tensor_reduce(
                            s1[:, p:p + 1], xT[:, p, :N],
                            axis=mybir.AxisListType.X, op=AL.add)
                    nc.scalar.activation(dummy[:], xT[:, p, :N], AF.Square,
                                         accum_out=s2[:, p:p + 1])
                # per-partition partials: s1r, s2r, gb.s1
                sc3 = wk.tile([128, 3], FP32, tag="sc3")
                nc.vector.tensor_reduce(sc3[:, 0:1], s1[:, :planes],
                                        axis=mybir.AxisListType.X, op=AL.add)
                nc.vector.tensor_reduce(sc3[:, 1:2], s2[:, :planes],
                                        axis=mybir.AxisListType.X, op=AL.add)
                gbs1 = wk.tile([128, 4], FP32, tag="gbs1")
                nc.vector.tensor_tensor(out=gbs1[:, :planes], in0=gb[:, :planes],
                                        in1=s1[:, :planes], op=AL.mult)
                nc.vector.tensor_reduce(sc3[:, 2:3], gbs1[:, :planes],
                                        axis=mybir.AxisListType.X, op=AL.add)
                pr = pst.tile([1, 4], FP32, tag="psT")
                nc.tensor.matmul(pr[:, :3], ones_colf[:], sc3[:], start=True,
                                 stop=True)
                # scalar land  (consts baked per call)
                NG = consts[f"NG{lni}"]; NG2 = consts[f"NG2{lni}"]
                invNC = 1.0 / (N * cout)
                sc = wk.tile([1, 8], FP32, tag="scal")
                # mu = (S1 + NG) * invNC
                nc.vector.tensor_scalar(out=sc[:, 0:1], in0=pr[:, 0:1],
                                        scalar1=float(NG), scalar2=invNC,
                                        op0=AL.add, op1=AL.mult)
                # ms = S2 + NG2 + 2*gbs1
                nc.vector.tensor_scalar(out=sc[:, 1:2], in0=pr[:, 1:2],
                                        scalar1=float(NG2), scalar2=None,
                                        op0=AL.add)
                nc.vector.scalar_tensor_tensor(
                    out=sc[:, 1:2], in0=pr[:, 2:3], scalar=2.0, in1=sc[:, 1:2],
                    op0=AL.mult, op1=AL.add)
                # var = ms*invNC - mu^2 ; inv = 1/sqrt(var+eps)
                nc.vector.tensor_tensor(out=sc[:, 2:3], in0=sc[:, 0:1],
                                        in1=sc[:, 0:1], op=AL.mult)
                nc.vector.tensor_scalar(out=sc[:, 1:2], in0=sc[:, 1:2],
                                        scalar1=invNC, scalar2=None,
                                        op0=AL.mult)
                nc.vector.tensor_tensor(out=sc[:, 2:3], in0=sc[:, 1:2],
                                        in1=sc[:, 2:3], op=AL.subtract)
                nc.vector.tensor_scalar(out=sc[:, 2:3], in0=sc[:, 2:3],
                                        scalar1=1e-5, scalar2=None, op0=AL.add)
                nc.scalar.activation(sc[:, 3:4], sc[:, 2:3], AF.Sqrt)
                nc.vector.reciprocal(sc[:, 4:5], sc[:, 3:4])
                mi = wk.tile([1, 2], FP32, tag="mi")
                nc.vector.tensor_copy(mi[:, 0:1], sc[:, 0:1])
                nc.vector.tensor_copy(mi[:, 1:2], sc[:, 4:5])
                pb = pst.tile([128, 2], FP32, tag="psT")
                nc.tensor.matmul(pb[:], ones_row[:], mi[:], start=True,
                                 stop=True)
                mu_c = wk.tile([128, 2], FP32, tag="muc")
                nc.vector.tensor_copy(mu_c[:], pb[:])
                # per-plane scale/bias then relu pass
                for p in range(planes):
                    s_col = wk.tile([128, 1], FP32, tag="scol")
                    nc.vector.tensor_tensor(out=s_col[:], in0=mu_c[:, 1:2],
                                            in1=lnw[:, p:p + 1], op=AL.mult)
                    t_col = wk.tile([128, 1], FP32, tag="tcol")
                    nc.vector.tensor_tensor(out=t_col[:], in0=gb[:, p:p + 1],
                                            in1=mu_c[:, 0:1], op=AL.subtract)
                    nc.vector.tensor_tensor(out=t_col[:], in0=t_col[:],
                                            in1=s_col[:], op=AL.mult)
                    nc.vector.tensor_tensor(out=t_col[:], in0=t_col[:],
                                            in1=lnb[:, p:p + 1], op=AL.add)
                    nc.scalar.activation(xT[:, p, :], xT[:, p, :], AF.Relu,
                                         bias=t_col[:], scale=s_col[:])

            # ---------- L0 ----------
            def l0_post(t, ps_agg, rec):
                xbar = wk.tile([128, 256], BF, tag="xbar")
                nc.vector.tensor_scalar(out=xbar[:], in0=ps_agg[:, :256],
                                        scalar1=rec[:], scalar2=None,
                                        op0=AL.mult)
                psh = psb.tile([128, 512], FP32, tag="psA")
                for kc in range(2):
                    pt = pst.tile([128, 128], BF, tag="psT")
                    nc.tensor.transpose(pt[:], xbar[:, kc * 128:(kc + 1) * 128],
                                        ident[:])
                    xbT = wk.tile([128, 128], BF, tag="xbT")
                    nc.vector.tensor_copy(xbT[:], pt[:])
                    nc.tensor.matmul(psh[:], xbT[:], g0W[:, kc, :],
                                     start=(kc == 0), stop=(kc == 1))
                outt = wk.tile([128, 512], BF, tag="outt")
                nc.scalar.activation(outt[:], psh[:], AF.Copy)
                rows = 128 if t < NT - 1 else 68
                nc.sync.dma_start(
                    coll_in[0][t * 128:t * 128 + rows, :], outt[:rows, :])

            if level >= 1:
                agg_phase(haug0, CAUG0, 256, l0_post)
            if level >= 2:
                co = exchange(0, 512)
                readback_ln(0, co, 512, 0)

            # ---------- L1 / L2 phase A from xT ----------
            def phase_A(Wd, wsdd, cin, cout, haug, caug):
                Wsb = wp.tile([128, 4, 512], BF, tag="Wsb")
                nc.sync.dma_start(Wsb[:, :cin // 128, :cout],
                                  Wd.rearrange("(a p) c -> p a c", p=128))
                wsd = wp.tile([128, 4, 2], BF, tag="wsd")
                nc.sync.dma_start(wsd[:, :cin // 128, :],
                                  wsdd.rearrange("(a p) c -> p a c", p=128))
                kcs = cin // 128
                for g4 in range(0, GT, 4):
                    nt4 = min(4, GT - g4)
                    hsb = wk.tile([128, 4, 640], BF, tag="hsb")
                    sdw = wk.tile([128, 4, 64], FP32, tag="sdw")
                    for i in range(nt4):
                        gt = g4 + i
                        ps = psb.tile([128, 512], FP32, tag="psA")
                        ps2 = pss.tile([128, 2], FP32, tag="psD")
                        for kc in range(kcs):
                            lhsT = xT[:, kc, gt * 128:(gt + 1) * 128]
                            nc.tensor.matmul(ps[:, :cout], lhsT,
                                             Wsb[:, kc, :cout],
                                             start=(kc == 0),
                                             stop=(kc == kcs - 1))
                            nc.tensor.matmul(ps2[:], lhsT, wsd[:, kc, :],
                                             start=(kc == 0),
                                             stop=(kc == kcs - 1))
                        if i % 2 == 0:
                            nc.scalar.activation(hsb[:, i, :cout],
                                                 ps[:, :cout], AF.Copy)
                        else:
                            nc.vector.tensor_copy(hsb[:, i, :cout],
                                                  ps[:, :cout])
                        nc.vector.tensor_copy(hsb[:, i, cout:cout + 1],
                                              ps2[:, 0:1])
                        nc.vector.tensor_copy(sdw[:, i, 0:1], ps2[:, 1:2])
                    nc.sync.dma_start(
                        haug[g4 * 128:(g4 + nt4) * 128, :].rearrange(
                            "(a p) c -> p a c", p=128), hsb[:, :nt4, :caug])
                    nc.sync.dma_start(
                        sdarr[g4 * 128:(g4 + nt4) * 128, :].rearrange(
                            "(a p) c -> p a c", p=128), sdw[:, :nt4, :])

            def mk_post(cval, dest):
                def post(t, ps_agg, rec):
                    if dest is None:
                        nc.vector.tensor_scalar(
                            out=out_q[:, t, :cval], in0=ps_agg[:, :cval],
                            scalar1=rec[:], scalar2=None, op0=AL.mult)
                    else:
                        outt = wk.tile([128, 512], BF, tag="outt")
                        nc.vector.tensor_scalar(
                            out=outt[:, :cval], in0=ps_agg[:, :cval],
                            scalar1=rec[:], scalar2=None, op0=AL.mult)
                        rows = 128 if t < NT - 1 else 68
                        nc.sync.dma_start(
                            dest[t * 128:t * 128 + rows, :cval],
                            outt[:rows, :cval])
                return post

            if level >= 3:
                phase_A(d_W1, d_wsd1, 512, 512, haug1, CAUG1)
            if level >= 4:
                agg_phase(haug1, CAUG1, 512, mk_post(512, coll_in[1]))
            if level >= 5:
                co = exchange(1, 512)
                readback_ln(1, co, 512, 1)
            if level >= 6:
                phase_A(d_W2, d_wsd2, 512, 256, haug2, CAUG2)
            if level >= 7:
                agg_phase(haug2, CAUG2, 256, mk_post(256, None))
            # out_q[:, :, :256] now holds h2 quarter (no exchange)

            # ---------- final: residual + FF x2 + mask ----------
            if level < 8:
                # still produce the declared output so compile succeeds
                for t in range(NT):
                    outf = wk.tile([128, 256], FP32, tag="outf")
                    nc.vector.tensor_scalar(out=outf[:], in0=x0q[:, t, :],
                                            scalar1=actq[:, t:t + 1],
                                            scalar2=None, op0=AL.mult)
                    nc.sync.dma_start(d_out[t * 128:(t + 1) * 128, :], outf[:])
                nc.compile()
                return nc
            xf = oqp.tile([128, NT, 256], BF)
            for t in range(NT):
                nc.vector.tensor_tensor(out=xf[:, t, :], in0=x0q[:, t, :],
                                        in1=out_q[:, t, :256], op=AL.add)
            for ff in range(2):
                for t in range(NT):
                    psh = psb.tile([128, 512], FP32, tag="psA")
                    for kc in range(2):
                        pt = pst.tile([128, 128], BF, tag="psT")
                        nc.tensor.transpose(
                            pt[:], xf[:, t, kc * 128:(kc + 1) * 128], ident[:])
                        xbT = wk.tile([128, 128], BF, tag="xbT")
                        nc.vector.tensor_copy(xbT[:], pt[:])
                        nc.tensor.matmul(psh[:, :256], xbT[:],
                                         ffW[:, 2 * ff + kc, :],
                                         start=(kc == 0), stop=False)
                    nc.tensor.matmul(psh[:, :256], e0[:],
                                     ffrow[:, ff * 256:(ff + 1) * 256],
                                     start=False, stop=True)
                    relu = wk.tile([128, 256], BF, tag="relu")
                    nc.scalar.activation(relu[:], psh[:, :256], AF.Relu)
                    nc.vector.tensor_tensor(out=xf[:, t, :], in0=xf[:, t, :],
                                            in1=relu[:], op=AL.add)
            for t in range(NT):
                outf = wk.tile([128, 256], FP32, tag="outf")
                nc.vector.tensor_scalar(out=outf[:], in0=xf[:, t, :],
                                        scalar1=actq[:, t:t + 1], scalar2=None,
                                        op0=AL.mult)
                nc.sync.dma_start(d_out[t * 128:(t + 1) * 128, :], outf[:])

    nc.compile()
    return nc


def kernel(**inputs):
    global _last_results, _last_nc, _last_in_maps
    from concourse.bass_utils import run_bass_kernel_spmd

    inp = {k: np.asarray(v) for k, v in inputs.items()}
    src = inp["src"].astype(np.int64)
    dst = inp["dst"].astype(np.int64)
    plans, nch = _build_plans(src, dst)
    caugs = (384, 640, 384)

    # host weight folds
    Wemb = inp["emb_W1"].astype(np.float64) @ inp["emb_W2"].astype(np.float64)
    bemb = (inp["emb_b1"].astype(np.float64) @ inp["emb_W2"].astype(np.float64)
            + inp["emb_b2"])
    ws0 = inp["g0_W"] @ inp["g0_as"]
    wd0 = inp["g0_W"] @ inp["g0_ad"]
    wnf = Wemb @ np.stack([ws0, wd0], 1).astype(np.float64)   # [7,2]
    bnf = bemb @ np.stack([ws0, wd0], 1).astype(np.float64)   # [2]
    wsd1 = np.stack([inp["g1_W"] @ inp["g1_as"], inp["g1_W"] @ inp["g1_ad"]], 1)
    wsd2 = np.stack([inp["g2_W"] @ inp["g2_as"], inp["g2_W"] @ inp["g2_ad"]], 1)

    consts = {}
    for li in (0, 1):
        gb = inp[f"g{li}_b"].astype(np.float64)
        consts[f"NG{li}"] = float(N * gb.sum())
        consts[f"NG2{li}"] = float(N * (gb * gb).sum())
    nc = _build_program(nch, caugs, consts)

    def pack_ln(li):
        cols = np.zeros((128, 12), np.float32)
        w = inp[f"ln{li}_w"]; b = inp[f"ln{li}_b"]; gb = inp[f"g{li}_b"]
        for p in range(4):
            cols[:, p] = w[p * 128:(p + 1) * 128]
            cols[:, 4 + p] = b[p * 128:(p + 1) * 128]
            cols[:, 8 + p] = gb[p * 128:(p + 1) * 128]
        return cols

    ln_all = np.concatenate([pack_ln(0), pack_ln(1)], 1)

    # per-core inputs
    nfg = [np.zeros((8, NPAD), np.float32) for _ in range(B)]
    for g in range(B):
        nfg[g][:7, :N] = inp["node_features"][g].T
        nfg[g][7, :N] = 1.0
    embrhs = np.concatenate(
        [np.concatenate([Wemb, wnf], 1),
         np.concatenate([bemb, bnf]).reshape(1, 258)], 0)      # [8,258]
    embq = np.concatenate(
        [Wemb, (bemb + inp["g2_b"].astype(np.float64)).reshape(1, 256)], 0)
    ffW = np.concatenate([inp["ff1_W"], inp["ff2_W"]], 0)      # [512,256]
    ffrow = np.concatenate([inp["ff1_b"], inp["ff2_b"]]).reshape(1, 512)

    in_maps = []
    for c in range(8):
        g, q = c // 4, c % 4
        pl = plans[c]
        actq = np.zeros((128, NT), np.float32)
        aq = inp["active_agents"][g][q * Q:(q + 1) * Q]
        for t in range(NT):
            seg = aq[t * 128:(t + 1) * 128]
            actq[:len(seg), t] = seg
        nfq = np.zeros((8, NT * 128), np.float32)
        nfq[:7, :Q] = inp["node_features"][g][q * Q:(q + 1) * Q].T
        nfq[7, :Q] = 1.0
        dl = pl["dstloc"].astype(np.float32).reshape(-1, 128).T.copy()
        in_maps.append({
            "nfT": _bf(nfg[g]), "nfqT": _bf(nfq),
            "embrhs": _bf(embrhs), "embqrow": _bf(embq),
            "W1": _bf(inp["g1_W"]), "W2": _bf(inp["g2_W"]),
            "wsd1": _bf(wsd1), "wsd2": _bf(wsd2), "g0W": _bf(inp["g0_W"]),
            "ln": np.ascontiguousarray(ln_all, np.float32).copy(),
            "ffW": _bf(ffW), "ffrow": _bf(ffrow),
            "isrc": _wrap16(pl["src_sorted"]), "idst": _wrap16(pl["dst_sorted"]),
            "dstloc": np.ascontiguousarray(dl),
            "actq": actq,
        })
    _last_nc, _last_in_maps = nc, in_maps
    globals()["_last_nch"], globals()["_last_consts"] = nch, consts
    res = run_bass_kernel_spmd(nc, in_maps, core_ids=list(range(8)))
    _last_results = res
    out = np.zeros((B, N, H), np.float32)
    for c in range(8):
        g, q = c // 4, c % 4
        out[g, q * Q:(q + 1) * Q] = res.results[c]["out"][:Q]
    return out, inp["active_agents"].astype(np.float32)
